# revision 3
# baseline (speedup 1.0000x reference)
"""ExpertScatter TRN2 kernel.

reference semantics:
    X = einsum('bekj,eji->beki', Y, W)          # per-head projection
    out[b] = zeros([T, I]); out[b, Ind[b,e,k]] += X[b,e,k]

Strategy (data-parallel over batch, 1 batch per NeuronCore):
  Phase A: per head e, matmul X_chunk[128 rows, 1024] = Yt_chunk.T @ W[e]
           (float32r matmuls = full PE rate with fp32 data), write X to an
           internal HBM staging buffer in natural row order.
  Host precomputes a global sort of the 16384 rows of each batch by target
  slot, padded to a fixed PT rows per 128-slot output tile.
  Phase B: per output tile (128 slots), dma_gather the contributing rows
           (PT of them) into SBUF, build one-hot selection matrices on DVE
           (is_equal against a column-iota constant), and accumulate
           out_tile = sum_g onehot_g.T @ Xrows_g in PSUM. One DMA per tile
           writes the finished [128, 1024] block of the output.

All shapes/counts are identical across cores (SPMD); per-core data
differences live entirely in the input tensors (Yt, gather indices,
relative-column tables).
"""

import os

import numpy as np

import concourse.bacc as bacc
import concourse.bass as bass
import concourse.mybir as mybir
import concourse.tile as tile
from concourse.bass_utils import run_bass_kernel_spmd

# Problem constants (hardcoded per harness contract).
B = 8
HEADS = 16
K = 1024
HEAD_DIM = 128
OUT_DIM = 1024
T_SLOTS = 4096

R = HEADS * K            # rows per batch = 16384
NT = T_SLOTS // 128      # output tiles per batch = 32
PT = 640                 # padded rows per output tile (max observed 565)
NG = PT // 128           # row groups (matmuls) per output tile = 5
NCORES = 8

F32 = mybir.dt.float32
F32R = mybir.dt.float32r
BF16 = mybir.dt.bfloat16
I16 = mybir.dt.int16

# Projection matmul dtype: float32r (full-rate fp32) or float32 (4x slower).
MM_F32R = os.environ.get("ES_MM_F32R", "1") == "1"
# X staging / scatter dtype: "f32r", "f32", or "bf16".
X_DTYPE = os.environ.get("ES_X_DTYPE", "f32r")

_cache = {}


def _build_program(mdt, sdt):
    """mdt: projection matmul dtype; sdt: X staging + scatter matmul dtype."""
    nc = bacc.Bacc("TRN2", target_bir_lowering=False, debug=False,
                   num_devices=NCORES)

    yt = nc.dram_tensor("yt", [HEAD_DIM, R], mdt, kind="ExternalInput").ap()
    w = nc.dram_tensor("w", [HEAD_DIM, HEADS * OUT_DIM], mdt,
                       kind="ExternalInput").ap()
    gidx = nc.dram_tensor("gidx", [128, NT * (PT // 16)], I16,
                          kind="ExternalInput").ap()
    relc = nc.dram_tensor("relc", [128, NT * NG], F32,
                          kind="ExternalInput").ap()
    cols = nc.dram_tensor("cols", [128, 128], F32, kind="ExternalInput").ap()
    out = nc.dram_tensor("out", [T_SLOTS, OUT_DIM], F32,
                         kind="ExternalOutput").ap()
    xnat = nc.dram_tensor("xnat", [R, OUT_DIM], sdt).ap()

    with tile.TileContext(nc) as tc:
        with (
            tc.tile_pool(name="const", bufs=1) as cpool,
            tc.tile_pool(name="yhead", bufs=2) as ypool,
            tc.tile_pool(name="xchunk", bufs=3) as xpool,
            tc.tile_pool(name="gather", bufs=2) as gpool,
            tc.tile_pool(name="onehot", bufs=4) as ohpool,
            tc.tile_pool(name="otile", bufs=2) as opool,
            tc.tile_pool(name="psum", bufs=2, space="PSUM") as pspool,
        ):
            w_sb = cpool.tile([128, HEADS * OUT_DIM], mdt, tag="w")
            nc.sync.dma_start(out=w_sb[:], in_=w[:])
            gidx_sb = cpool.tile([128, NT * (PT // 16)], I16, tag="gidx")
            nc.sync.dma_start(out=gidx_sb[:], in_=gidx[:])
            relc_sb = cpool.tile([128, NT * NG], F32, tag="relc")
            nc.sync.dma_start(out=relc_sb[:], in_=relc[:])
            cols_sb = cpool.tile([128, 128], F32, tag="cols")
            nc.sync.dma_start(out=cols_sb[:], in_=cols[:])

            # ---- Phase A: projection, X written to HBM in natural order --
            for e in range(HEADS):
                yt_e = ypool.tile([128, K], mdt, tag="yt")
                nc.sync.dma_start(out=yt_e[:], in_=yt[:, e * K:(e + 1) * K])
                for rc in range(K // 128):
                    px = pspool.tile([128, OUT_DIM], F32, tag="pa")
                    lhsT = yt_e[:, rc * 128:(rc + 1) * 128]
                    for h in range(2):
                        nc.tensor.matmul(
                            out=px[:, h * 512:(h + 1) * 512],
                            lhsT=lhsT,
                            rhs=w_sb[:, e * OUT_DIM + h * 512:
                                     e * OUT_DIM + (h + 1) * 512],
                            start=True, stop=True,
                        )
                    xc = xpool.tile([128, OUT_DIM], sdt, tag="xc")
                    nc.vector.tensor_copy(out=xc[:], in_=px[:])
                    row0 = (e * (K // 128) + rc) * 128
                    nc.sync.dma_start(out=xnat[row0:row0 + 128, :], in_=xc[:])

            # Fence: every gather below reads rows written above.
            tc.strict_bb_all_engine_barrier()

            # ---- Phase B: gather sorted rows per tile, one-hot matmul ----
            for t in range(NT):
                g = gpool.tile([128, NG, OUT_DIM], sdt, tag="g")
                nc.gpsimd.dma_gather(
                    out_ap=g[:],
                    in_ap=xnat[:],
                    idxs_ap=gidx_sb[:, t * (PT // 16):(t + 1) * (PT // 16)],
                    num_idxs=PT,
                    num_idxs_reg=PT,
                    elem_size=OUT_DIM,
                )
                pt = pspool.tile([128, OUT_DIM], F32, tag="pb")
                for gi in range(NG):
                    oh = ohpool.tile([128, 128], sdt, tag="oh")
                    c = t * NG + gi
                    nc.vector.tensor_tensor(
                        out=oh[:],
                        in0=relc_sb[:, c:c + 1].to_broadcast([128, 128]),
                        in1=cols_sb[:],
                        op=mybir.AluOpType.is_equal,
                    )
                    for h in range(2):
                        nc.tensor.matmul(
                            out=pt[:, h * 512:(h + 1) * 512],
                            lhsT=oh[:],
                            rhs=g[:, gi, h * 512:(h + 1) * 512],
                            start=(gi == 0), stop=(gi == NG - 1),
                        )
                ot = opool.tile([128, OUT_DIM], F32, tag="ot")
                nc.vector.tensor_copy(out=ot[:], in_=pt[:])
                nc.sync.dma_start(out=out[t * 128:(t + 1) * 128, :], in_=ot[:])

    nc.compile()
    return nc


def _get_program():
    mdt = F32R if MM_F32R else F32
    sdt = {"f32r": F32R if MM_F32R else F32, "f32": F32, "bf16": BF16}[X_DTYPE]
    key = (MM_F32R, X_DTYPE)
    if key not in _cache:
        _cache[key] = _build_program(mdt, sdt)
    return _cache[key]


def _prep_core_inputs(Yb, Indb):
    """Host-side prep for one batch: transpose Y, sort rows by slot,
    build padded gather-index and relative-column tables."""
    yt = np.ascontiguousarray(
        Yb.transpose(2, 0, 1).reshape(HEAD_DIM, R)).astype(np.float32)
    ind = Indb.reshape(R).astype(np.int64)
    order = np.argsort(ind, kind="stable")
    sind = ind[order]
    counts = np.bincount(sind // 128, minlength=NT)
    assert counts.max() <= PT, f"tile overflow: {counts.max()} > {PT}"
    gidx = np.zeros((NT, PT), dtype=np.int16)       # padded w/ row 0
    relc = np.full((NT, PT), -1000.0, dtype=np.float32)
    pos = 0
    for t in range(NT):
        c = counts[t]
        gidx[t, :c] = order[pos:pos + c]
        relc[t, :c] = (sind[pos:pos + c] - t * 128).astype(np.float32)
        pos += c
    # dma_gather index layout: position p -> (partition p%16, col p//16),
    # and the 16-partition block replicated across all 8 Q7 core groups.
    blk = np.concatenate(
        [gidx[t].reshape(PT // 16, 16).T for t in range(NT)], axis=1)
    gidx_sb = np.ascontiguousarray(np.tile(blk, (8, 1)), dtype=np.int16)
    # one-hot layout: position p -> (partition p%128, group p//128)
    relc_sb = np.concatenate(
        [relc[t].reshape(NG, 128).T for t in range(NT)], axis=1)
    relc_sb = np.ascontiguousarray(relc_sb, dtype=np.float32)
    return yt, gidx_sb, relc_sb


def kernel(Y, Ind, T, W):
    Y = np.asarray(Y, dtype=np.float32)
    Ind = np.asarray(Ind)
    W = np.asarray(W, dtype=np.float32)
    assert int(T) == T_SLOTS and Y.shape == (B, HEADS, K, HEAD_DIM)

    nc = _get_program()

    w_in = np.ascontiguousarray(
        W.transpose(1, 0, 2).reshape(HEAD_DIM, HEADS * OUT_DIM)
    ).astype(np.float32)
    cols_in = np.broadcast_to(
        np.arange(128, dtype=np.float32)[None, :], (128, 128)).copy()

    in_maps = []
    for b in range(B):
        yt, gidx_sb, relc_sb = _prep_core_inputs(Y[b], Ind[b])
        in_maps.append({
            "yt": yt, "w": w_in, "gidx": gidx_sb,
            "relc": relc_sb, "cols": cols_in,
        })

    res = run_bass_kernel_spmd(
        nc, in_maps, core_ids=list(range(NCORES)),
        trace=os.environ.get("ES_TRACE", "0") == "1",
    )
    kernel.last_results = res
    out = np.stack([res.results[b]["out"] for b in range(B)], axis=0)
    return out
